# revision 24
# baseline (speedup 1.0000x reference)
"""CSA sparse attention Trainium2 kernel.

Sharding: 8 cores = 2 batches x 4 head-groups (4 heads each).
Each core computes its batch's partial output projection for its 4 heads;
host sums the 4 partials per batch and adds bo.

Per core (T=C=1024, hd=64, 4 local heads), head-PAIR-major schedule:
  Prologue: Q/K projections (f32) for pair 0 only, K_compT + column norms
  (Sqrt + DVE reciprocal) + KnT for pair 0, then R(0..3) and the bf16
  V/V_comp stage; the pair-1 projections and K_comp run as background PE
  quanta interleaved into the first ~12 main-loop iterations.  Inputs are
  loaded as few large multi-chunk DMAs split across the SP and ACT
  hardware DGE queues (the queue dispatch, ~0.6us per DMA, is the scarce
  resource, not bandwidth).
  Main loop per (head, t-tile): theta = exact 64th largest of R[t,:] via
  top-8 per 64-chunk (16 max8/match_replace) + top-8 per 512-half
  remainder + rank-64 merge over 144 candidates (8 max8 + 7 match_replace)
  -- the DVE runs ONLY this selection chain plus the deferred row-norm
  reciprocals.  The top-64 mask is applied additively: Pool computes
  m = (R < theta) * -30000 (bf16), which is injected into the score PSUM
  via a transposing matmul against the identity (start=True) before the
  bf16 score matmul accumulates on top (start=False); one ACT exp then
  yields the masked exp'd scores directly.  Attention out + rowsum use a
  ones column of V_comp; row normalization (DVE reciprocal + ACT scaled
  copy) is deferred 6 iterations to keep the DVE FIFO free of stalls.
"""

import numpy as np

T = 1024
DM = 1024
C = 1024
HD = 64
HPC = 4              # heads per core
DLOC = HPC * HD      # 256
NCH = DM // 128      # 8 contraction chunks
NTT = T // 128       # 8 t-tiles
NEG = -1.0e30

_NC = None


def build_nc():
    import concourse.bass as bass
    import concourse.bacc as bacc
    import concourse.mybir as mybir
    from concourse.tile import TileContext
    from concourse.masks import make_identity

    F32 = mybir.dt.float32
    BF16 = mybir.dt.bfloat16
    AF = mybir.ActivationFunctionType
    ALU = mybir.AluOpType

    nc = bacc.Bacc("TRN2", target_bir_lowering=False, debug=False, num_devices=8)

    xT = nc.dram_tensor("xT", [DM, T], F32, kind="ExternalInput")
    xTb = nc.dram_tensor("xTb", [DM, T], BF16, kind="ExternalInput")
    wqT = nc.dram_tensor("wqT", [DM, DLOC], F32, kind="ExternalInput")
    wkT = nc.dram_tensor("wkT", [DM, DLOC], F32, kind="ExternalInput")
    wvTb = nc.dram_tensor("wvTb", [DM, DLOC], BF16, kind="ExternalInput")
    wcT = nc.dram_tensor("wcT", [T, C], F32, kind="ExternalInput")
    wcTb = nc.dram_tensor("wcTb", [T, C], BF16, kind="ExternalInput")
    woTb = nc.dram_tensor("woTb", [DLOC, DM], BF16, kind="ExternalInput")
    outp = nc.dram_tensor("outp", [T, DM], F32, kind="ExternalOutput")

    with TileContext(nc) as tc:
        from contextlib import ExitStack
        with ExitStack() as ctx:
            const = ctx.enter_context(tc.tile_pool(name="const", bufs=1))
            res = ctx.enter_context(tc.tile_pool(name="res", bufs=1))
            stream = ctx.enter_context(tc.tile_pool(name="stream", bufs=4))
            lw = ctx.enter_context(tc.tile_pool(name="lw", bufs=2))
            rsp = ctx.enter_context(tc.tile_pool(name="rsp", bufs=5))

            # ---- constants ----
            identb = const.tile([128, 128], BF16, tag="identb")
            make_identity(nc, identb[:])
            hsel = const.tile([128, 2], F32, tag="hsel")
            nc.vector.memset(hsel[:], 0.0)
            nc.vector.memset(hsel[0:64, 0:1], 1.0)
            nc.vector.memset(hsel[64:128, 1:2], 1.0)
            identf = const.tile([128, 128], F32, tag="identf")
            make_identity(nc, identf[:])
            ones2 = const.tile([2, 128], F32, tag="ones2")
            # prime ACT function tables early
            prim = const.tile([1, 8], F32, tag="prim")
            nc.vector.memset(prim[:], 1.0)
            for fn_ in (AF.Square, AF.Sqrt, AF.Exp):
                nc.scalar.activation(prim[:], prim[:], fn_)

            # ---- resident tensors ----
            wq_sb = res.tile([128, NCH, DLOC], F32, tag="wq_sb")
            wk_sb = res.tile([128, NCH, DLOC], F32, tag="wk_sb")
            wv_sb = res.tile([128, NCH, DLOC], BF16, tag="wv_sb")
            wo_sb = res.tile([128, 2, DM], BF16, tag="wo_sb")
            wct_sb = res.tile([128, NCH, C], F32, tag="wct_sb")
            qt = res.tile([128, 2, T], F32, tag="qt")
            qtb = res.tile([128, 2, T], BF16, tag="qtb")
            k_sb = res.tile([128, NTT, DLOC], F32, tag="k_sb")
            v_sb = res.tile([128, NTT, DLOC], BF16, tag="v_sb")
            kct = res.tile([128, 2, C], F32, tag="kct")
            kctb = res.tile([128, 2, C], BF16, tag="kctb")
            knt = res.tile([128, 2, C], F32, tag="knt")
            vca = res.tile([128, NCH, HPC * 65], BF16, tag="vca")
            attn = res.tile([128, NTT, DLOC], BF16, tag="attn")
            aoT = res.tile([128, 2, T], BF16, tag="aoT")
            sqt = res.tile([128, 512], F32, tag="sqt")

            # weight + compress-matrix loads up front (overlap with AB0).
            # SP queue: AB-critical loads; ACT queue: wct/CE traffic.
            nc.sync.dma_start(
                wq_sb[:], wqT[:].rearrange("(c p) f -> p c f", c=NCH, p=128))
            nc.sync.dma_start(
                wk_sb[:], wkT[:].rearrange("(c p) f -> p c f", c=NCH, p=128))

            # pair-major iteration order: heads {0,1} for all t-tiles, then {2,3}
            ITERS = [(h, tt) for hp in range(2) for tt in range(NTT)
                     for h in (2 * hp, 2 * hp + 1)]
            NIT = len(ITERS)
            rs_t = {}
            ao_t = {}

            def AB_quanta(hp, pool):
                """Q/K projections for head-pair hp; PSUM groups are per-bank,
                so the four K t-block accumulations run j-outer over resident
                x chunks."""
                for tb in range(2):
                    pq1 = pool.tile([128, 512], F32, tag="pq1", name=f"pq{hp}_{tb}")
                    pk1 = pool.tile([128, 512], F32, tag="pk1", name=f"pk{hp}_{tb}")
                    xt_a = stream.tile([128, NCH, 512], F32, tag="xta", bufs=1,
                                       name=f"xta{hp}_{tb}")
                    for hh in range(2):
                        nc.sync.dma_start(
                            xt_a[:, hh * 4:(hh + 1) * 4, :],
                            xT[hh * 512:(hh + 1) * 512,
                               tb * 512:(tb + 1) * 512].rearrange(
                                   "(c p) f -> p c f", c=4, p=128))
                    for ch in range(NCH):
                        nc.tensor.matmul(
                            pq1[:], lhsT=wq_sb[:, ch, hp * 128:(hp + 1) * 128],
                            rhs=xt_a[:, ch, :], start=(ch == 0), stop=(ch == NCH - 1))
                        yield
                    nc.scalar.activation(
                        qt[:, hp, tb * 512:(tb + 1) * 512], pq1[:], AF.Copy)
                    for j in range(4):
                        for ch in range(NCH):
                            nc.tensor.matmul(
                                pk1[:, j * 128:(j + 1) * 128],
                                lhsT=xt_a[:, ch, j * 128:(j + 1) * 128],
                                rhs=wk_sb[:, ch, hp * 128:(hp + 1) * 128],
                                start=(ch == 0), stop=(ch == NCH - 1))
                        nc.scalar.activation(
                            k_sb[:, tb * 4 + j, hp * 128:(hp + 1) * 128],
                            pk1[:, j * 128:(j + 1) * 128], AF.Copy)
                        yield

            def D_quanta(pr, prp, pst):
                """K_compT, column norms (Rsqrt) and KnT for pair pr."""
                for cb in range(2):
                    cbs = slice(cb * 512, (cb + 1) * 512)
                    pkc = prp.tile([128, 512], F32, tag="rtag", name=f"pkc{pr}_{cb}")
                    for ch in range(NCH):
                        nc.tensor.matmul(
                            pkc[:], lhsT=k_sb[:, ch, pr * 128:(pr + 1) * 128],
                            rhs=wct_sb[:, ch, cbs],
                            start=(ch == 0), stop=(ch == NCH - 1))
                        if ch % 4 == 3:
                            yield
                    nc.scalar.activation(kct[:, pr, cbs], pkc[:], AF.Copy)
                    nc.scalar.activation(sqt[:], kct[:, pr, cbs], AF.Square)
                    pn = pst.tile([128, 512], F32, tag="pstt", name=f"pn{pr}_{cb}")
                    nc.tensor.matmul(
                        pn[0:2, :], lhsT=hsel[:], rhs=sqt[:],
                        start=True, stop=True)
                    nrm2 = stream.tile([2, 512], F32, tag="iv", name=f"iv{pr}_{cb}")
                    nc.scalar.activation(nrm2[:], pn[0:2, :], AF.Sqrt)
                    invk2 = stream.tile([2, 512], F32, tag="iv2", name=f"iv2{pr}_{cb}")
                    nc.vector.reciprocal(invk2[:], nrm2[:])
                    pb = pst.tile([128, 512], F32, tag="pstt", name=f"pb{pr}_{cb}")
                    nc.tensor.matmul(
                        pb[:], lhsT=ones2[:], rhs=invk2[:], start=True, stop=True)
                    pbs = stream.tile([128, 512], F32, tag="pbs", name=f"pbs{pr}_{cb}")
                    nc.scalar.activation(pbs[:], pb[:], AF.Copy)
                    nc.gpsimd.tensor_mul(knt[:, pr, cbs], kct[:, pr, cbs], pbs[:])
                    nc.scalar.activation(qtb[:, pr, cbs], qt[:, pr, cbs], AF.Copy)
                    nc.scalar.activation(kctb[:, pr, cbs], kct[:, pr, cbs], AF.Copy)
                    yield

            def emit_R(i, prp):
                h, tt = ITERS[i]
                dt_, sub = h // 2, (h % 2) * 64
                rs = rsp.tile([128, C], F32, tag="rs", name=f"rs{i}")
                rs_t[i] = rs
                for cb in range(2):
                    psr = prp.tile([128, 512], F32, tag="rtag", name=f"psr{i}_{cb}")
                    nc.tensor.matmul(
                        psr[:],
                        lhsT=qt[sub:sub + 64, dt_, tt * 128:(tt + 1) * 128],
                        rhs=knt[sub:sub + 64, dt_, cb * 512:(cb + 1) * 512],
                        start=True, stop=True)
                    nc.scalar.activation(
                        rs[:, cb * 512:(cb + 1) * 512], psr[:], AF.Copy)

            # ---- open loop-region psum pools ----
            with tc.tile_pool(name="prp", bufs=2, space="PSUM") as prp, \
                 tc.tile_pool(name="pst", bufs=1, space="PSUM") as pst, \
                 tc.tile_pool(name="pao", bufs=1, space="PSUM") as pao, \
                 tc.tile_pool(name="pce", bufs=1, space="PSUM") as pce, \
                 tc.tile_pool(name="pabx", bufs=1, space="PSUM") as pabx:

                # ones2 = hsel.T (per-sub-head broadcast selectors)
                ptons = pst.tile([128, 512], F32, tag="pstt", name="onesT")
                nc.tensor.transpose(ptons[0:2, 0:128], hsel[:], identf[:])
                nc.scalar.activation(ones2[:], ptons[0:2, 0:128], AF.Copy)

                # prologue: pair-0 projections + K_comp/norms, then R(0..3)
                for _ in AB_quanta(0, pabx):
                    pass
                for hh in range(2):
                    nc.scalar.dma_start(
                        wct_sb[:, hh * 4:(hh + 1) * 4, :],
                        wcT[hh * 512:(hh + 1) * 512, :].rearrange(
                            "(c p) f -> p c f", c=4, p=128))
                for _ in D_quanta(0, prp, pst):
                    pass
                for i in range(4):
                    emit_R(i, prp)

                def STAGE_CE():
                    nc.scalar.dma_start(
                        wv_sb[:], wvTb[:].rearrange("(c p) f -> p c f", c=NCH, p=128))
                    nc.scalar.dma_start(
                        wo_sb[:], woTb[:].rearrange("(c p) f -> p c f", c=2, p=128))
                    # V projection (bf16)
                    for tb in range(2):
                        for jp in range(2):
                            pv = pce.tile([128, 512], F32, tag="pv",
                                          name=f"pv{tb}_{jp}")
                            xb_a = stream.tile([128, NCH, 256], BF16, tag="xba",
                                               bufs=1, name=f"xba{tb}_{jp}")
                            nc.scalar.dma_start(
                                xb_a[:],
                                xTb[:, tb * 512 + jp * 256:
                                    tb * 512 + (jp + 1) * 256].rearrange(
                                        "(c p) f -> p c f", c=NCH, p=128))
                            for j2 in range(2):
                                for ch in range(NCH):
                                    nc.tensor.matmul(
                                        pv[:, j2 * 256:(j2 + 1) * 256],
                                        lhsT=xb_a[:, ch, j2 * 128:(j2 + 1) * 128],
                                        rhs=wv_sb[:, ch, :],
                                        start=(ch == 0), stop=(ch == NCH - 1))
                                nc.scalar.activation(
                                    v_sb[:, tb * 4 + jp * 2 + j2, :],
                                    pv[:, j2 * 256:(j2 + 1) * 256], AF.Copy)
                    # V_comp (bf16) + ones column
                    pvE = pce.tile([128, 512], F32, tag="pv", name="pvE")
                    for ct in range(NCH):
                        pvc = pvE[:, (ct % 2) * 256:(ct % 2 + 1) * 256]
                        wcb = stream.tile([128, NCH, 128], BF16, tag="wcb",
                                          bufs=2, name=f"wcb{ct}")
                        nc.scalar.dma_start(
                            wcb[:],
                            wcTb[:, ct * 128:(ct + 1) * 128].rearrange(
                                "(c p) f -> p c f", c=NCH, p=128))
                        for ch in range(NCH):
                            nc.tensor.matmul(
                                pvc, lhsT=wcb[:, ch, :], rhs=v_sb[:, ch, :],
                                start=(ch == 0), stop=(ch == NCH - 1))
                        nc.gpsimd.memset(vca[:, ct, :], 1.0)
                        for h in range(HPC):
                            nc.scalar.activation(
                                vca[:, ct, h * 65:h * 65 + 64],
                                pvc[:, h * 64:(h + 1) * 64], AF.Copy)

                def emit_tail(i):
                    h, tt = ITERS[i]
                    dt_, sub = h // 2, (h % 2) * 64
                    rs = rs_t.pop(i)
                    # --- exact top-64 threshold (DVE only) ---
                    cr = lw.tile([128, 144], F32, tag="cr", name=f"cr{i}")
                    rz = lw.tile([128, C], F32, tag="rz", name=f"rz{i}")
                    for kc in range(16):
                        sl = rs[:, kc * 64:(kc + 1) * 64]
                        zl = rz[:, kc * 64:(kc + 1) * 64]
                        c0 = cr[:, kc * 8:kc * 8 + 8]
                        nc.vector.max(c0, sl)
                        nc.vector.match_replace(
                            zl, in_to_replace=c0, in_values=sl, imm_value=NEG)
                    for hf in range(2):
                        nc.vector.max(cr[:, 128 + hf * 8:136 + hf * 8],
                                      rz[:, hf * 512:(hf + 1) * 512])
                    maxs = lw.tile([128, 64], F32, tag="maxs", name=f"maxs{i}")
                    for r in range(8):
                        nc.vector.max(maxs[:, r * 8:(r + 1) * 8], cr[:])
                        if r < 7:
                            nc.vector.match_replace(
                                cr[:], in_to_replace=maxs[:, r * 8:(r + 1) * 8],
                                in_values=cr[:], imm_value=NEG)
                    theta = maxs[:, 63:64]
                    # --- additive mask (Pool): 0 keep / -30000 drop ---
                    m = lw.tile([128, C], BF16, tag="m", name=f"m{i}")
                    nc.gpsimd.tensor_scalar(
                        m[:], rs[:], theta, -30000.0, op0=ALU.is_lt, op1=ALU.mult)
                    # --- scores with mask injected into PSUM; exp on ACT ---
                    et = lw.tile([128, C], BF16, tag="et", name=f"et{i}")
                    for half in range(2):
                        ph = pst.tile([128, 512], F32, tag="pstt",
                                      name=f"ps{i}_{half}")
                        for q in range(4):
                            ct = half * 4 + q
                            nc.tensor.matmul(
                                ph[:, q * 128:(q + 1) * 128],
                                lhsT=m[:, ct * 128:(ct + 1) * 128],
                                rhs=identb[:],
                                start=True, stop=False)
                            nc.tensor.matmul(
                                ph[:, q * 128:(q + 1) * 128],
                                lhsT=kctb[sub:sub + 64, dt_, ct * 128:(ct + 1) * 128],
                                rhs=qtb[sub:sub + 64, dt_, tt * 128:(tt + 1) * 128],
                                start=False, stop=True)
                        nc.scalar.activation(
                            et[:, half * 512:(half + 1) * 512], ph[:],
                            AF.Exp, scale=0.125)
                    # --- attention output + rowsum via ones column ---
                    ao = pao.tile([128, 65], F32, tag="ao", name=f"ao{i}")
                    for ct in range(8):
                        nc.tensor.matmul(
                            ao[:], lhsT=et[:, ct * 128:(ct + 1) * 128],
                            rhs=vca[:, ct, h * 65:(h + 1) * 65],
                            start=(ct == 0), stop=(ct == 7))
                    aos = lw.tile([128, 65], F32, tag="aos", bufs=8, name=f"aos{i}")
                    nc.scalar.activation(aos[:], ao[:], AF.Copy)
                    ao_t[i] = aos

                def emit_final_tt(tt):
                    ftile = pce.tile([128, 512], F32, tag="pv", name=f"ptr{tt}")
                    ptr2 = ftile[:, 256:384].bitcast(BF16)
                    for dc in range(2):
                        nc.tensor.transpose(
                            ptr2[:, dc * 128:(dc + 1) * 128],
                            attn[:, tt, dc * 128:(dc + 1) * 128], identb[:])
                    nc.scalar.activation(
                        aoT[:, 0:2, tt * 128:(tt + 1) * 128], ptr2[:], AF.Copy)
                    for q in range(4):
                        po = pce.tile([128, 512], F32, tag="pv", name=f"po{tt}_{q}")
                        for dc in range(2):
                            nc.tensor.matmul(
                                po[:, 0:256], lhsT=aoT[:, dc, tt * 128:(tt + 1) * 128],
                                rhs=wo_sb[:, dc, q * 256:(q + 1) * 256],
                                start=(dc == 0), stop=(dc == 1))
                        osb = lw.tile([128, 256], F32, tag="osb", bufs=4,
                                      name=f"osb{tt}_{q}")
                        nc.scalar.activation(osb[:], po[:, 0:256], AF.Copy)
                        nc.sync.dma_start(
                            outp[tt * 128:(tt + 1) * 128, q * 256:(q + 1) * 256],
                            osb[:])

                def emit_norm(i):
                    h, tt = ITERS[i]
                    aos = ao_t.pop(i)
                    rec = lw.tile([128, 1], F32, tag="rec", name=f"rec{i}")
                    nc.vector.reciprocal(rec[:], aos[:, 64:65])
                    nc.scalar.activation(
                        attn[:, tt, h * 64:(h + 1) * 64], aos[:, 0:64],
                        AF.Copy, scale=rec[:])
                    if h == 3:
                        emit_final_tt(tt)

                STAGE_CE()

                # background pair-1 prologue, interleaved as PE quanta
                import itertools
                bg = itertools.chain(AB_quanta(1, pabx), D_quanta(1, prp, pst))

                for i in range(NIT):
                    for _ in range(3):
                        next(bg, None)
                    if i + 4 < NIT:
                        emit_R(i + 4, prp)
                    emit_tail(i)
                    if i >= 6:
                        emit_norm(i - 6)
                for i in range(NIT - 6, NIT):
                    emit_norm(i)

    nc.compile()
    return nc


def _get_nc():
    global _NC
    if _NC is None:
        _NC = build_nc()
    return _NC


def make_in_maps(inputs):
    import ml_dtypes
    x = np.asarray(inputs["x"], np.float32)
    Wq = np.asarray(inputs["Wq"], np.float32)
    Wk = np.asarray(inputs["Wk"], np.float32)
    Wv = np.asarray(inputs["Wv"], np.float32)
    Wo = np.asarray(inputs["Wo"], np.float32)
    Wc = np.asarray(inputs["Wc"], np.float32)
    wcT = np.ascontiguousarray(Wc.T)
    wcTb = wcT.astype(ml_dtypes.bfloat16)
    in_maps = []
    for core in range(8):
        b, g = core // 4, core % 4
        sl = slice(g * DLOC, (g + 1) * DLOC)
        xTf = np.ascontiguousarray(x[b].T)
        in_maps.append(dict(
            xT=xTf,
            xTb=xTf.astype(ml_dtypes.bfloat16),
            wqT=np.ascontiguousarray(Wq[sl, :].T),
            wkT=np.ascontiguousarray(Wk[sl, :].T),
            wvTb=np.ascontiguousarray(Wv[sl, :].T).astype(ml_dtypes.bfloat16),
            wcT=wcT,
            wcTb=wcTb,
            woTb=np.ascontiguousarray(Wo[:, sl].T).astype(ml_dtypes.bfloat16),
        ))
    return in_maps


def kernel(**inputs):
    from concourse.bass_utils import run_bass_kernel_spmd
    in_maps = make_in_maps(inputs)
    r = run_bass_kernel_spmd(_get_nc(), in_maps, core_ids=list(range(8)))
    outs = [res["outp"] for res in r.results]
    out = np.zeros((2, T, DM), np.float32)
    for core in range(8):
        out[core // 4] += outs[core]
    out += np.asarray(inputs["bo"], np.float32)[None, None, :]
    return out


# revision 26
# speedup vs baseline: 1.0435x; 1.0435x over previous
"""CSA sparse attention Trainium2 kernel.

Sharding: 8 cores = 2 batches x 4 head-groups (4 heads each).
Each core computes its batch's partial output projection for its 4 heads;
host sums the 4 partials per batch and adds bo.

Per core (T=C=1024, hd=64, 4 local heads), head-PAIR-major schedule:
  Prologue: Q/K projections (f32) for pair 0 only, K_compT + column norms
  (Sqrt + DVE reciprocal) + KnT for pair 0, then R(0..3) and the bf16
  V/V_comp stage; the pair-1 projections and K_comp run as background PE
  quanta interleaved into the first ~12 main-loop iterations.  Inputs are
  loaded as few large multi-chunk DMAs split across the SP and ACT
  hardware DGE queues (the queue dispatch, ~0.6us per DMA, is the scarce
  resource, not bandwidth).
  Main loop per (head, t-tile): theta = exact 64th largest of R[t,:] via
  top-8 per 64-chunk (16 max8/match_replace) + top-8 per 512-half
  remainder + rank-64 merge over 144 candidates (8 max8 + 7 match_replace)
  -- the DVE runs ONLY this selection chain plus the deferred row-norm
  reciprocals.  The top-64 mask is applied additively: Pool computes
  m = (R < theta) * -30000 (bf16), which is injected into the score PSUM
  via a transposing matmul against the identity (start=True) before the
  bf16 score matmul accumulates on top (start=False); one ACT exp then
  yields the masked exp'd scores directly.  Attention out + rowsum use a
  ones column of V_comp; row normalization (DVE reciprocal + ACT scaled
  copy) is deferred 6 iterations to keep the DVE FIFO free of stalls.
"""

import numpy as np

T = 1024
DM = 1024
C = 1024
HD = 64
HPC = 4              # heads per core
DLOC = HPC * HD      # 256
NCH = DM // 128      # 8 contraction chunks
NTT = T // 128       # 8 t-tiles
NEG = -1.0e30

_NC = None


def build_nc():
    import concourse.bass as bass
    import concourse.bacc as bacc
    import concourse.mybir as mybir
    from concourse.tile import TileContext
    from concourse.masks import make_identity

    F32 = mybir.dt.float32
    BF16 = mybir.dt.bfloat16
    AF = mybir.ActivationFunctionType
    ALU = mybir.AluOpType

    nc = bacc.Bacc("TRN2", target_bir_lowering=False, debug=False, num_devices=8)

    xT = nc.dram_tensor("xT", [DM, T], F32, kind="ExternalInput")
    xTb = nc.dram_tensor("xTb", [DM, T], BF16, kind="ExternalInput")
    wqT = nc.dram_tensor("wqT", [DM, DLOC], F32, kind="ExternalInput")
    wkT = nc.dram_tensor("wkT", [DM, DLOC], F32, kind="ExternalInput")
    wvTb = nc.dram_tensor("wvTb", [DM, DLOC], BF16, kind="ExternalInput")
    wcT = nc.dram_tensor("wcT", [T, C], F32, kind="ExternalInput")
    wcTb = nc.dram_tensor("wcTb", [T, C], BF16, kind="ExternalInput")
    woTb = nc.dram_tensor("woTb", [DLOC, DM], BF16, kind="ExternalInput")
    outp = nc.dram_tensor("outp", [T, DM], F32, kind="ExternalOutput")

    with TileContext(nc) as tc:
        from contextlib import ExitStack
        with ExitStack() as ctx:
            const = ctx.enter_context(tc.tile_pool(name="const", bufs=1))
            res = ctx.enter_context(tc.tile_pool(name="res", bufs=1))
            stream = ctx.enter_context(tc.tile_pool(name="stream", bufs=4))
            lw = ctx.enter_context(tc.tile_pool(name="lw", bufs=2))
            rsp = ctx.enter_context(tc.tile_pool(name="rsp", bufs=5))

            # ---- constants ----
            identb = const.tile([128, 128], BF16, tag="identb")
            make_identity(nc, identb[:])
            hsel = const.tile([128, 2], F32, tag="hsel")
            nc.vector.memset(hsel[:], 0.0)
            nc.vector.memset(hsel[0:64, 0:1], 1.0)
            nc.vector.memset(hsel[64:128, 1:2], 1.0)
            identf = const.tile([128, 128], F32, tag="identf")
            make_identity(nc, identf[:])
            ones2 = const.tile([2, 128], F32, tag="ones2")
            # prime ACT function tables early
            prim = const.tile([1, 8], F32, tag="prim")
            nc.vector.memset(prim[:], 1.0)
            for fn_ in (AF.Square, AF.Sqrt, AF.Exp):
                nc.scalar.activation(prim[:], prim[:], fn_)

            # ---- resident tensors ----
            wq_sb = res.tile([128, NCH, DLOC], F32, tag="wq_sb")
            wk_sb = res.tile([128, NCH, DLOC], F32, tag="wk_sb")
            wv_sb = res.tile([128, NCH, DLOC], BF16, tag="wv_sb")
            wo_sb = res.tile([128, 2, DM], BF16, tag="wo_sb")
            wct_sb = res.tile([128, NCH, C], F32, tag="wct_sb")
            qt = res.tile([128, 2, T], F32, tag="qt")
            qtb = res.tile([128, 2, T], BF16, tag="qtb")
            k_sb = res.tile([128, NTT, DLOC], F32, tag="k_sb")
            v_sb = res.tile([128, NTT, DLOC], BF16, tag="v_sb")
            kct = res.tile([128, 2, C], F32, tag="kct")
            kctb = res.tile([128, 2, C], BF16, tag="kctb")
            knt = res.tile([128, 2, C], F32, tag="knt")
            vca = res.tile([128, NCH, HPC * 65], BF16, tag="vca")
            attn = res.tile([128, NTT, DLOC], BF16, tag="attn")
            aoT = res.tile([128, 2, T], BF16, tag="aoT")
            sqt = res.tile([128, 512], F32, tag="sqt")

            # weight + compress-matrix loads up front (overlap with AB0).
            # SP queue: AB-critical loads; ACT queue: wct/CE traffic.
            nc.sync.dma_start(
                wq_sb[:], wqT[:].rearrange("(c p) f -> p c f", c=NCH, p=128))
            nc.sync.dma_start(
                wk_sb[:], wkT[:].rearrange("(c p) f -> p c f", c=NCH, p=128))

            # pair-major iteration order: heads {0,1} for all t-tiles, then {2,3}
            ITERS = [(h, tt) for hp in range(2) for tt in range(NTT)
                     for h in (2 * hp, 2 * hp + 1)]
            NIT = len(ITERS)
            rs_t = {}
            ao_t = {}

            def AB_quanta(hp, pool):
                """Q/K projections for head-pair hp; PSUM groups are per-bank,
                so the four K t-block accumulations run j-outer over resident
                x chunks."""
                for tb in range(2):
                    pq1 = pool.tile([128, 512], F32, tag="pq1", name=f"pq{hp}_{tb}")
                    pk1 = pool.tile([128, 512], F32, tag="pk1", name=f"pk{hp}_{tb}")
                    xt_a = stream.tile([128, NCH, 512], F32, tag="xta", bufs=1,
                                       name=f"xta{hp}_{tb}")
                    for hh in range(2):
                        nc.sync.dma_start(
                            xt_a[:, hh * 4:(hh + 1) * 4, :],
                            xT[hh * 512:(hh + 1) * 512,
                               tb * 512:(tb + 1) * 512].rearrange(
                                   "(c p) f -> p c f", c=4, p=128))
                    for j in range(4):
                        for ch in range(NCH):
                            nc.tensor.matmul(
                                pk1[:, j * 128:(j + 1) * 128],
                                lhsT=xt_a[:, ch, j * 128:(j + 1) * 128],
                                rhs=wk_sb[:, ch, hp * 128:(hp + 1) * 128],
                                start=(ch == 0), stop=(ch == NCH - 1))
                        nc.scalar.activation(
                            k_sb[:, tb * 4 + j, hp * 128:(hp + 1) * 128],
                            pk1[:, j * 128:(j + 1) * 128], AF.Copy)
                        yield
                    for ch in range(NCH):
                        nc.tensor.matmul(
                            pq1[:], lhsT=wq_sb[:, ch, hp * 128:(hp + 1) * 128],
                            rhs=xt_a[:, ch, :], start=(ch == 0), stop=(ch == NCH - 1))
                        yield
                    nc.scalar.activation(
                        qt[:, hp, tb * 512:(tb + 1) * 512], pq1[:], AF.Copy)

            def D_quanta(pr, prp, pst):
                """K_compT, column norms (Rsqrt) and KnT for pair pr."""
                for cb in range(2):
                    cbs = slice(cb * 512, (cb + 1) * 512)
                    pkc = prp.tile([128, 512], F32, tag="rtag", name=f"pkc{pr}_{cb}")
                    for ch in range(NCH):
                        nc.tensor.matmul(
                            pkc[:], lhsT=k_sb[:, ch, pr * 128:(pr + 1) * 128],
                            rhs=wct_sb[:, ch, cbs],
                            start=(ch == 0), stop=(ch == NCH - 1))
                        if ch % 4 == 3:
                            yield
                    nc.scalar.activation(kct[:, pr, cbs], pkc[:], AF.Copy)
                    nc.scalar.activation(sqt[:], kct[:, pr, cbs], AF.Square)
                    pn = pst.tile([128, 512], F32, tag="pstt", name=f"pn{pr}_{cb}")
                    nc.tensor.matmul(
                        pn[0:2, :], lhsT=hsel[:], rhs=sqt[:],
                        start=True, stop=True)
                    nrm2 = stream.tile([2, 512], F32, tag="iv", name=f"iv{pr}_{cb}")
                    nc.scalar.activation(nrm2[:], pn[0:2, :], AF.Sqrt)
                    invk2 = stream.tile([2, 512], F32, tag="iv2", name=f"iv2{pr}_{cb}")
                    nc.vector.reciprocal(invk2[:], nrm2[:])
                    pb = pst.tile([128, 512], F32, tag="pstt", name=f"pb{pr}_{cb}")
                    nc.tensor.matmul(
                        pb[:], lhsT=ones2[:], rhs=invk2[:], start=True, stop=True)
                    pbs = stream.tile([128, 512], F32, tag="pbs", name=f"pbs{pr}_{cb}")
                    nc.scalar.activation(pbs[:], pb[:], AF.Copy)
                    nc.gpsimd.tensor_mul(knt[:, pr, cbs], kct[:, pr, cbs], pbs[:])
                    nc.scalar.activation(qtb[:, pr, cbs], qt[:, pr, cbs], AF.Copy)
                    nc.scalar.activation(kctb[:, pr, cbs], kct[:, pr, cbs], AF.Copy)
                    yield

            def emit_R(i, prp):
                h, tt = ITERS[i]
                dt_, sub = h // 2, (h % 2) * 64
                rs = rsp.tile([128, C], F32, tag="rs", name=f"rs{i}")
                rs_t[i] = rs
                for cb in range(2):
                    psr = prp.tile([128, 512], F32, tag="rtag", name=f"psr{i}_{cb}")
                    nc.tensor.matmul(
                        psr[:],
                        lhsT=qt[sub:sub + 64, dt_, tt * 128:(tt + 1) * 128],
                        rhs=knt[sub:sub + 64, dt_, cb * 512:(cb + 1) * 512],
                        start=True, stop=True)
                    nc.scalar.activation(
                        rs[:, cb * 512:(cb + 1) * 512], psr[:], AF.Copy)

            # ---- open loop-region psum pools ----
            with tc.tile_pool(name="prp", bufs=2, space="PSUM") as prp, \
                 tc.tile_pool(name="pst", bufs=1, space="PSUM") as pst, \
                 tc.tile_pool(name="pao", bufs=1, space="PSUM") as pao, \
                 tc.tile_pool(name="pce", bufs=1, space="PSUM") as pce, \
                 tc.tile_pool(name="pabx", bufs=1, space="PSUM") as pabx:

                # ones2 = hsel.T (per-sub-head broadcast selectors)
                ptons = pst.tile([128, 512], F32, tag="pstt", name="onesT")
                nc.tensor.transpose(ptons[0:2, 0:128], hsel[:], identf[:])
                nc.scalar.activation(ones2[:], ptons[0:2, 0:128], AF.Copy)

                # prologue: pair-0 projections + K_comp/norms, then R(0..3)
                for _ in AB_quanta(0, pabx):
                    pass
                for hh in range(2):
                    nc.scalar.dma_start(
                        wct_sb[:, hh * 4:(hh + 1) * 4, :],
                        wcT[hh * 512:(hh + 1) * 512, :].rearrange(
                            "(c p) f -> p c f", c=4, p=128))
                for _ in D_quanta(0, prp, pst):
                    pass
                for i in range(4):
                    emit_R(i, prp)

                def STAGE_CE():
                    nc.scalar.dma_start(
                        wv_sb[:], wvTb[:].rearrange("(c p) f -> p c f", c=NCH, p=128))
                    nc.scalar.dma_start(
                        wo_sb[:], woTb[:].rearrange("(c p) f -> p c f", c=2, p=128))
                    # V projection (bf16)
                    for tb in range(2):
                        for jp in range(2):
                            pv = pce.tile([128, 512], F32, tag="pv",
                                          name=f"pv{tb}_{jp}")
                            xb_a = stream.tile([128, NCH, 256], BF16, tag="xba",
                                               bufs=1, name=f"xba{tb}_{jp}")
                            nc.scalar.dma_start(
                                xb_a[:],
                                xTb[:, tb * 512 + jp * 256:
                                    tb * 512 + (jp + 1) * 256].rearrange(
                                        "(c p) f -> p c f", c=NCH, p=128))
                            for j2 in range(2):
                                for ch in range(NCH):
                                    nc.tensor.matmul(
                                        pv[:, j2 * 256:(j2 + 1) * 256],
                                        lhsT=xb_a[:, ch, j2 * 128:(j2 + 1) * 128],
                                        rhs=wv_sb[:, ch, :],
                                        start=(ch == 0), stop=(ch == NCH - 1))
                                nc.scalar.activation(
                                    v_sb[:, tb * 4 + jp * 2 + j2, :],
                                    pv[:, j2 * 256:(j2 + 1) * 256], AF.Copy)
                    # V_comp (bf16) + ones column
                    pvE = pce.tile([128, 512], F32, tag="pv", name="pvE")
                    for ct in range(NCH):
                        pvc = pvE[:, (ct % 2) * 256:(ct % 2 + 1) * 256]
                        wcb = stream.tile([128, NCH, 128], BF16, tag="wcb",
                                          bufs=2, name=f"wcb{ct}")
                        nc.scalar.dma_start(
                            wcb[:],
                            wcTb[:, ct * 128:(ct + 1) * 128].rearrange(
                                "(c p) f -> p c f", c=NCH, p=128))
                        for ch in range(NCH):
                            nc.tensor.matmul(
                                pvc, lhsT=wcb[:, ch, :], rhs=v_sb[:, ch, :],
                                start=(ch == 0), stop=(ch == NCH - 1))
                        nc.gpsimd.memset(vca[:, ct, :], 1.0)
                        for h in range(HPC):
                            nc.scalar.activation(
                                vca[:, ct, h * 65:h * 65 + 64],
                                pvc[:, h * 64:(h + 1) * 64], AF.Copy)

                def emit_tail(i):
                    h, tt = ITERS[i]
                    dt_, sub = h // 2, (h % 2) * 64
                    rs = rs_t.pop(i)
                    # --- exact top-64 threshold (DVE only) ---
                    cr = lw.tile([128, 144], F32, tag="cr", name=f"cr{i}")
                    rz = lw.tile([128, C], F32, tag="rz", name=f"rz{i}")
                    for kc in range(16):
                        sl = rs[:, kc * 64:(kc + 1) * 64]
                        zl = rz[:, kc * 64:(kc + 1) * 64]
                        c0 = cr[:, kc * 8:kc * 8 + 8]
                        nc.vector.max(c0, sl)
                        nc.vector.match_replace(
                            zl, in_to_replace=c0, in_values=sl, imm_value=NEG)
                    for hf in range(2):
                        nc.vector.max(cr[:, 128 + hf * 8:136 + hf * 8],
                                      rz[:, hf * 512:(hf + 1) * 512])
                    maxs = lw.tile([128, 64], F32, tag="maxs", name=f"maxs{i}")
                    for r in range(8):
                        nc.vector.max(maxs[:, r * 8:(r + 1) * 8], cr[:])
                        if r < 7:
                            nc.vector.match_replace(
                                cr[:], in_to_replace=maxs[:, r * 8:(r + 1) * 8],
                                in_values=cr[:], imm_value=NEG)
                    theta = maxs[:, 63:64]
                    # --- additive mask (Pool): 0 keep / -30000 drop ---
                    m = lw.tile([128, C], BF16, tag="m", name=f"m{i}")
                    nc.gpsimd.tensor_scalar(
                        m[:], rs[:], theta, -30000.0, op0=ALU.is_lt, op1=ALU.mult)
                    # --- scores with mask injected into PSUM; exp on ACT ---
                    et = lw.tile([128, C], BF16, tag="et", name=f"et{i}")
                    for half in range(2):
                        ph = pst.tile([128, 512], F32, tag="pstt",
                                      name=f"ps{i}_{half}")
                        for q in range(4):
                            ct = half * 4 + q
                            nc.tensor.matmul(
                                ph[:, q * 128:(q + 1) * 128],
                                lhsT=m[:, ct * 128:(ct + 1) * 128],
                                rhs=identb[:],
                                start=True, stop=False)
                            nc.tensor.matmul(
                                ph[:, q * 128:(q + 1) * 128],
                                lhsT=kctb[sub:sub + 64, dt_, ct * 128:(ct + 1) * 128],
                                rhs=qtb[sub:sub + 64, dt_, tt * 128:(tt + 1) * 128],
                                start=False, stop=True)
                        nc.scalar.activation(
                            et[:, half * 512:(half + 1) * 512], ph[:],
                            AF.Exp, scale=0.125)
                    # --- attention output + rowsum via ones column ---
                    ao = pao.tile([128, 65], F32, tag="ao", name=f"ao{i}")
                    for ct in range(8):
                        nc.tensor.matmul(
                            ao[:], lhsT=et[:, ct * 128:(ct + 1) * 128],
                            rhs=vca[:, ct, h * 65:(h + 1) * 65],
                            start=(ct == 0), stop=(ct == 7))
                    aos = lw.tile([128, 65], F32, tag="aos", bufs=8, name=f"aos{i}")
                    nc.scalar.activation(aos[:], ao[:], AF.Copy)
                    ao_t[i] = aos

                def emit_final_tt(tt):
                    ftile = pce.tile([128, 512], F32, tag="pv", name=f"ptr{tt}")
                    ptr2 = ftile[:, 256:384].bitcast(BF16)
                    for dc in range(2):
                        nc.tensor.transpose(
                            ptr2[:, dc * 128:(dc + 1) * 128],
                            attn[:, tt, dc * 128:(dc + 1) * 128], identb[:])
                    nc.scalar.activation(
                        aoT[:, 0:2, tt * 128:(tt + 1) * 128], ptr2[:], AF.Copy)
                    for q in range(4):
                        po = pce.tile([128, 512], F32, tag="pv", name=f"po{tt}_{q}")
                        for dc in range(2):
                            nc.tensor.matmul(
                                po[:, 0:256], lhsT=aoT[:, dc, tt * 128:(tt + 1) * 128],
                                rhs=wo_sb[:, dc, q * 256:(q + 1) * 256],
                                start=(dc == 0), stop=(dc == 1))
                        osb = lw.tile([128, 256], F32, tag="osb", bufs=4,
                                      name=f"osb{tt}_{q}")
                        nc.scalar.activation(osb[:], po[:, 0:256], AF.Copy)
                        nc.sync.dma_start(
                            outp[tt * 128:(tt + 1) * 128, q * 256:(q + 1) * 256],
                            osb[:])

                def emit_norm(i):
                    h, tt = ITERS[i]
                    aos = ao_t.pop(i)
                    rec = lw.tile([128, 1], F32, tag="rec", name=f"rec{i}")
                    nc.vector.reciprocal(rec[:], aos[:, 64:65])
                    nc.scalar.activation(
                        attn[:, tt, h * 64:(h + 1) * 64], aos[:, 0:64],
                        AF.Copy, scale=rec[:])
                    if h == 3:
                        emit_final_tt(tt)

                STAGE_CE()
                next_norm = [0]

                # background pair-1 prologue, interleaved as PE quanta
                import itertools
                bg = itertools.chain(AB_quanta(1, pabx), D_quanta(1, prp, pst))

                for i in range(NIT):
                    for _ in range(3):
                        next(bg, None)
                    if i + 4 < NIT:
                        emit_R(i + 4, prp)
                    emit_tail(i)
                    lim = i - 3 if i >= 28 else i - 6
                    while next_norm[0] <= lim:
                        emit_norm(next_norm[0])
                        next_norm[0] += 1
                for i in range(next_norm[0], NIT):
                    emit_norm(i)

    nc.compile()
    return nc


def _get_nc():
    global _NC
    if _NC is None:
        _NC = build_nc()
    return _NC


def make_in_maps(inputs):
    import ml_dtypes
    x = np.asarray(inputs["x"], np.float32)
    Wq = np.asarray(inputs["Wq"], np.float32)
    Wk = np.asarray(inputs["Wk"], np.float32)
    Wv = np.asarray(inputs["Wv"], np.float32)
    Wo = np.asarray(inputs["Wo"], np.float32)
    Wc = np.asarray(inputs["Wc"], np.float32)
    wcT = np.ascontiguousarray(Wc.T)
    wcTb = wcT.astype(ml_dtypes.bfloat16)
    in_maps = []
    for core in range(8):
        b, g = core // 4, core % 4
        sl = slice(g * DLOC, (g + 1) * DLOC)
        xTf = np.ascontiguousarray(x[b].T)
        in_maps.append(dict(
            xT=xTf,
            xTb=xTf.astype(ml_dtypes.bfloat16),
            wqT=np.ascontiguousarray(Wq[sl, :].T),
            wkT=np.ascontiguousarray(Wk[sl, :].T),
            wvTb=np.ascontiguousarray(Wv[sl, :].T).astype(ml_dtypes.bfloat16),
            wcT=wcT,
            wcTb=wcTb,
            woTb=np.ascontiguousarray(Wo[:, sl].T).astype(ml_dtypes.bfloat16),
        ))
    return in_maps


def kernel(**inputs):
    from concourse.bass_utils import run_bass_kernel_spmd
    in_maps = make_in_maps(inputs)
    r = run_bass_kernel_spmd(_get_nc(), in_maps, core_ids=list(range(8)))
    outs = [res["outp"] for res in r.results]
    out = np.zeros((2, T, DM), np.float32)
    for core in range(8):
        out[core // 4] += outs[core]
    out += np.asarray(inputs["bo"], np.float32)[None, None, :]
    return out
